# revision 25
# baseline (speedup 1.0000x reference)
"""Trainium2 Bass kernel for EnhancedKANLayer (spline-order-3 KAN layer).

Reference computation (fp32):
    x_norm = tanh(x[:, None, :] / scaler[None, :, :])          # (B, O, I)
    d      = |x_norm[..., None] - grid|                        # (B, O, I, G)
    b      = exp(-d**3);  bhat = b / (sum_g b + 1e-8)
    out    = einsum('boig,oig->bo', bhat, W) + bias

With scaler uniform across O (as produced by setup_inputs), x_norm is
O-independent.  The G=8 normalized basis functions bhat_g(t) are fixed
smooth scalar functions of t = tanh(x) on (-1, 1), so we replace them by
a degree-D polynomial fit (Chebyshev fit, converted to monomial basis;
coefficients are small so monomials are bf16-safe):

    bhat_g(t) ~= sum_k c[k,g] t^k
    out[b,o]  = sum_{i,k} t_{bi}^k A[o,i,k] + bias_eff[o]
    A[o,i,k]  = sum_g c[k,g] W[o,i,g],  bias_eff = bias + sum_i A[:,i,0]

This kills the whole elementwise basis pipeline (sub/abs/square/mult/
exp/reduce/recip/normalize over B*I*G elements) and leaves: one tanh,
a handful of bf16 power products over B*I elements, and K'*2 small
accumulating matmuls.  Fit error at deg 6 gives end-to-end rel ~3.6e-3
(measured against the jax reference; bf16 matmul floor is ~2.4e-3).

Sharding: data-parallel over batch across 8 NeuronCores (64 rows/core,
A replicated).  Per core, raw-bacc program (manual semaphores):
  SYNC:   x DMA (two column-halves = the two i-chunks), out DMA
  SCALAR: weight half-a DMA (k=1,2), ACT table prefetch dummy, tanh
  DVE:    ones memset, x2/x3/x4/P6 bf16 products, psum->sbuf copy
  GPSIMD: weight half-b DMA (k>=3 + bias rows), P5 products
  PE:     13 accumulating bf16 matmuls (K'=6 powers x 2 i-chunks +
          one 2-row Kahan-split bias matmul vs a ones vector)
Bias is applied exactly via the two-row bf16 Kahan split (hi+lo).
Falls back to a pure-numpy reference path if scaler is not uniform
across O or shapes differ (never hit by the real input distribution).
"""

import os
import sys
import types

import numpy as np

N_CORES = 8
B, I, O, G = 512, 256, 128, 8
NCH = I // 128             # i-chunks of 128 partitions (2)
EPS = 1e-8


def _grid():
    """Shard grid: BQ batch-shards x OQ out-shards (BQ*OQ = 8)."""
    g = os.environ.get("NKERN_GRID", "4x2")
    bq, oq = (int(v) for v in g.split("x"))
    assert bq * oq == N_CORES
    return bq, oq

_CACHE = {}
_FIT_CACHE = {}


def _ensure_axon_ntff_hook():
    """Register the NTFF profiling hook (missing antenv.axon_hooks shim).
    Only needed for traced runs; harmless otherwise."""
    try:
        import antenv
        if 'antenv.axon_hooks' not in sys.modules:
            mod = types.ModuleType('antenv.axon_hooks')
            holder = [None]
            mod.set_axon_ntff_profile_hook = lambda h: holder.__setitem__(0, h)
            mod.get_axon_ntff_profile_hook = lambda: holder[0]
            sys.modules['antenv.axon_hooks'] = mod
            antenv.axon_hooks = mod
        mod = sys.modules['antenv.axon_hooks']
        if mod.get_axon_ntff_profile_hook() is None:
            from trn_agent_boot.trn_boot import _ntff_profile_via_ctypes
            so = '/opt/axon/libaxon_pjrt.so'
            if os.path.exists(so):
                mod.set_axon_ntff_profile_hook(_ntff_profile_via_ctypes(so))
    except Exception:
        pass


def _reference_numpy(x, spline_weight, spline_scaler, bias, grid_points):
    """General fallback, mirrors the jax reference in numpy (fp32)."""
    x = x.astype(np.float32)
    xn = np.tanh(x[:, None, :] / spline_scaler[None, :, :])          # (B,O,I)
    d = np.abs(xn[..., None] - grid_points)                           # (B,O,I,G)
    b = np.exp(-(d ** 3))
    bhat = b / (b.sum(axis=-1, keepdims=True) + EPS)
    out = np.einsum('boig,oig->bo', bhat, spline_weight, optimize=True)
    return (out + bias[None, :]).astype(np.float32)


def _fit_mono(grid_points, deg):
    """Chebyshev-fit the G normalized basis functions on t in [-1,1],
    return monomial coefficients mono[k, g] (k = 0..deg)."""
    key = (grid_points.tobytes(), deg)
    if key in _FIT_CACHE:
        return _FIT_CACHE[key]
    import numpy.polynomial.chebyshev as C
    g = grid_points.astype(np.float64)
    ts = np.cos(np.pi * (np.arange(4000) + 0.5) / 4000)
    d = np.abs(ts[:, None] - g[None, :])
    b = np.exp(-(d ** 3))
    bh = b / (b.sum(-1, keepdims=True) + EPS)
    mono = np.stack(
        [C.cheb2poly(C.chebfit(ts, bh[:, j], deg)) for j in range(len(g))],
        axis=1)                                                  # (deg+1, G)
    _FIT_CACHE[key] = mono
    return mono


def _build_program(deg, waitout):
    """Raw bacc program for the polynomial-KAN kernel; deg+1 = K powers.

    Power products (all bf16, halves h = i-chunk):
      T = tanh(x)          [ACT]
      x2 = T*T, x3 = x2*T, x4 = x2*x2, P6 = x3*x3   [DVE]
      P5 = x2*x3           [GPSIMD]
      (deg 7 adds P7 = x3*x4 on GPSIMD; deg 5 drops P6)
    """
    from contextlib import ExitStack

    from concourse import bacc, mybir

    f32 = mybir.dt.float32
    f16 = mybir.dt.float16
    AF = mybir.ActivationFunctionType
    ALU = mybir.AluOpType

    KP = deg                     # number of non-constant powers (k = 1..KP)
    assert 4 <= KP <= 7
    BQ, OQ = _grid()
    BSH, OSH = B // BQ, O // OQ  # per-core batch rows / out cols
    XC = NCH * BSH               # x-tile cols (ch, b)
    WA = 2 * 2 * OSH             # k=1,2 cols (both chunks) -> Sync queue
    if KP == 5:
        # k3 + bias -> Scalar queue; k4,k5 -> GpSimd queue
        WB = WA + 2 * OSH + OSH
    else:
        # k3,k4 + bias -> Scalar queue; k>=5 -> GpSimd queue
        WB = WA + (2 if KP >= 4 else 1) * 2 * OSH + OSH
    WCOLS = KP * 2 * OSH + OSH   # + bias block (2 Kahan rows x OSH o)

    nc = bacc.Bacc("TRN2", target_bir_lowering=False, debug=False,
                   num_devices=N_CORES)

    x_d = nc.dram_tensor("x", [128, XC], f16, kind="ExternalInput")
    wr_d = nc.dram_tensor("wr", [128, WCOLS], f16, kind="ExternalInput")
    out_d = nc.dram_tensor("out", [BSH, OSH], f32, kind="ExternalOutput")

    def wcol(k, ch):
        base = ((k - 1) * 2 + ch) * OSH
        if KP == 5:
            return base + (OSH if k >= 4 else 0)   # bias sits before k4
        return base + (OSH if k >= 5 else 0)       # bias sits before k5

    bias_col = 3 * 2 * OSH if KP == 5 else (
        4 * 2 * OSH if KP >= 5 else KP * 2 * OSH)

    with ExitStack() as ctx:
        e = ctx.enter_context
        xs = e(nc.sbuf_tensor([128, XC], f16))
        T = e(nc.sbuf_tensor([128, XC], f16))
        x2 = e(nc.sbuf_tensor([128, XC], f16))
        x3 = e(nc.sbuf_tensor([128, XC], f16))
        x4 = e(nc.sbuf_tensor([128, XC], f16))
        P5 = e(nc.sbuf_tensor([128, XC], f16))
        P6 = e(nc.sbuf_tensor([128, XC], f16))
        P7 = e(nc.sbuf_tensor([128, XC], f16))
        wr = e(nc.sbuf_tensor([128, WCOLS], f16))
        ones = e(nc.sbuf_tensor([2, BSH], f16))
        jnkt = e(nc.sbuf_tensor([2, 256], f16))
        scr = e(nc.psum_tensor([BSH, 256], f32))
        outsb = e(nc.sbuf_tensor([BSH, OSH], f32))
        psum = e(nc.psum_tensor([BSH, OSH], f32))

        dmaX0 = e(nc.semaphore("dmaX0"))
        dmaX1 = e(nc.semaphore("dmaX1"))
        dmaWa = e(nc.semaphore("dmaWa"))
        dmaWb = e(nc.semaphore("dmaWb"))
        dmaWc = e(nc.semaphore("dmaWc"))
        dmaO = e(nc.semaphore("dmaO"))
        sOnes = e(nc.semaphore("sOnes"))
        sT = e(nc.semaphore("sT"))
        s2 = e(nc.semaphore("s2"))
        s3 = e(nc.semaphore("s3"))
        s4 = e(nc.semaphore("s4"))
        s5 = e(nc.semaphore("s5"))
        s6 = e(nc.semaphore("s6"))
        s7 = e(nc.semaphore("s7"))
        sP = e(nc.semaphore("sP"))
        sC = e(nc.semaphore("sC"))

        def hs(h):
            return slice(h * BSH, (h + 1) * BSH)

        dmaX = (dmaX0, dmaX1)

        n_warm = int(os.environ.get("NKERN_WARM", "7"))
        outq = os.environ.get("NKERN_OUTQ", "sync")
        use_block = bool(int(os.environ.get("NKERN_BLOCK", "0")))

        # All instructions are emitted flat into the main bb (no Block):
        # no per-engine entry branches, no exit drains, no exit barrier —
        # the NRT epilogue synchronizes and drains anyway.
        nc.sync.dma_start(xs[:, hs(0)], x_d.ap()[:, hs(0)],
                          single_packet=True).then_inc(dmaX0, 16)
        nc.scalar.dma_start(xs[:, hs(1)], x_d.ap()[:, hs(1)],
                            single_packet=True).then_inc(dmaX1, 16)
        nc.sync.dma_start(wr[:, 0:WA], wr_d.ap()[:, 0:WA]).then_inc(dmaWa, 16)
        nc.scalar.dma_start(wr[:, WA:WB],
                            wr_d.ap()[:, WA:WB]).then_inc(dmaWb, 16)
        nc.gpsimd.dma_start(wr[:, WB:WCOLS],
                            wr_d.ap()[:, WB:WCOLS]).then_inc(dmaWc, 16)

        if use_block:
            block = e(nc.Block(no_gpsimd_drain=True))
            sync_sec = block.sync
            scalar_sec = block.scalar
            vector_sec = block.vector
            gpsimd_sec = block.gpsimd
            tensor_sec = block.tensor
        else:
            def _flat(f):
                return lambda g: g(f)
            sync_sec = lambda f: f(nc.sync)
            scalar_sec = lambda f: f(nc.scalar)
            vector_sec = lambda f: f(nc.vector)
            gpsimd_sec = lambda f: f(nc.gpsimd)
            tensor_sec = lambda f: f(nc.tensor)

        @sync_sec
        def _(sync):
            if outq == "sync":
                sync.wait_ge(sC, 1)
                sync.dma_start(out_d.ap(), outsb[:]).then_inc(dmaO, 16)
                if waitout:
                    sync.wait_ge(dmaO, 16)

        @scalar_sec
        def _(scalar):
            # (no dummy ACT needed: the ACT_TABLE_LOAD pseudo-inst hoists to
            # the scalar stream start, well before x lands)
            for h in range(2):
                scalar.wait_ge(dmaX[h], 16)
                nc.scalar.activation(T[:, hs(h)], xs[:, hs(h)],
                                     AF.Tanh).then_inc(sT, 1)
            if KP == 5:
                scalar.wait_ge(s2, 1)
                nc.scalar.activation(x4[:, hs(0)], x2[:, hs(0)],
                                     AF.Square).then_inc(s4, 1)

        @vector_sec
        def _(vector):
            nc.vector.memset(ones[:], 1.0).then_inc(sOnes, 1)
            if n_warm:
                nc.vector.memset(jnkt[:], 0.5).then_inc(sOnes, 1)
            if KP == 5:
                for h in range(2):
                    vector.wait_ge(sT, h + 1)
                    nc.vector.tensor_tensor(x2[:, hs(h)], T[:, hs(h)],
                                            T[:, hs(h)],
                                            op=ALU.mult).then_inc(s2, 1)
                    nc.vector.tensor_tensor(x3[:, hs(h)], x2[:, hs(h)],
                                            T[:, hs(h)],
                                            op=ALU.mult).then_inc(s3, 1)
                for h in range(2):
                    nc.vector.tensor_tensor(P5[:, hs(h)], x2[:, hs(h)],
                                            x3[:, hs(h)],
                                            op=ALU.mult).then_inc(s5, 1)
            else:
                for h in range(2):
                    vector.wait_ge(sT, h + 1)
                    nc.vector.tensor_tensor(x2[:, hs(h)], T[:, hs(h)],
                                            T[:, hs(h)],
                                            op=ALU.mult).then_inc(s2, 1)
                    nc.vector.tensor_tensor(x3[:, hs(h)], x2[:, hs(h)],
                                            T[:, hs(h)],
                                            op=ALU.mult).then_inc(s3, 1)
                    if KP >= 4:
                        nc.vector.tensor_tensor(x4[:, hs(h)], x2[:, hs(h)],
                                                x2[:, hs(h)],
                                                op=ALU.mult).then_inc(s4, 1)
                    if KP >= 6:
                        nc.vector.tensor_tensor(P6[:, hs(h)], x3[:, hs(h)],
                                                x3[:, hs(h)],
                                                op=ALU.mult).then_inc(s6, 1)
            vector.wait_ge(sP, 1)
            nc.vector.tensor_scalar(outsb[:], psum[:], 1.0, None,
                                    op0=ALU.mult).then_inc(sC, 1)

        @gpsimd_sec
        def _(gpsimd):
            if KP == 5:
                gpsimd.wait_ge(s2, 2)
                nc.gpsimd.tensor_tensor(x4[:, hs(1)], x2[:, hs(1)],
                                        x2[:, hs(1)],
                                        op=ALU.mult).then_inc(s7, 1)
            else:
                for h in range(2):
                    if KP >= 5:
                        gpsimd.wait_ge(s3, h + 1)
                        nc.gpsimd.tensor_tensor(P5[:, hs(h)], x2[:, hs(h)],
                                                x3[:, hs(h)],
                                                op=ALU.mult).then_inc(s5, 1)
                    if KP >= 7:
                        gpsimd.wait_ge(s4, h + 1)
                        nc.gpsimd.tensor_tensor(P7[:, hs(h)], x3[:, hs(h)],
                                                x4[:, hs(h)],
                                                op=ALU.mult).then_inc(s7, 1)

        pw = {1: T, 2: x2, 3: x3, 4: x4, 5: P5, 6: P6, 7: P7}
        if KP == 5:
            # x4_0 from ACT (s4), x4_1 from GPSIMD (s7): separate sems
            psem = {(1, 0): (sT, 1), (1, 1): (sT, 2),
                    (2, 0): (s2, 1), (2, 1): (s2, 2),
                    (3, 0): (s3, 1), (3, 1): (s3, 2),
                    (4, 0): (s4, 1), (4, 1): (s7, 1),
                    (5, 0): (s5, 1), (5, 1): (s5, 2)}
        else:
            psem = {(k, ch): (s, ch + 1) for k, s in
                    {1: sT, 2: s2, 3: s3, 4: s4, 5: s5, 6: s6, 7: s7}.items()
                    for ch in range(2)}

        @tensor_sec
        def _(tensor):
            # three weight groups, each consumed in chunk-then-k order as
            # its DMA lands; k=0 denotes the 2-row Kahan bias matmul
            grp_a = [(1, 0), (1, 1), (2, 0), (2, 1)]
            if KP == 5:
                # k4 last: its second half comes from the slow GPSIMD mult
                grp_b = [(3, 0), (3, 1), (0, 0)]
                grp_c = [(5, 0), (5, 1), (4, 0), (4, 1)]
            else:
                grp_b = ([(k, ch) for k in range(3, min(KP, 4) + 1)
                          for ch in range(2)] + [(0, 0)])
                grp_c = [(k, ch) for k in range(5, KP + 1)
                         for ch in range(2)]
            waited = {}

            def emit(k, ch, start, stop):
                if k == 0:
                    tensor.wait_ge(sOnes, 1)
                    return nc.tensor.matmul(
                        psum[:], ones[:], wr[0:2, bias_col:bias_col + OSH],
                        start=start, stop=stop)
                sem, need = psem[(k, ch)]
                if waited.get(id(sem), 0) < need:
                    tensor.wait_ge(sem, need)
                    waited[id(sem)] = need
                return nc.tensor.matmul(
                    psum[:], pw[k][:, hs(ch)],
                    wr[:, wcol(k, ch):wcol(k, ch) + OSH],
                    start=start, stop=stop)

            # p-state warmup: junk matmuls on a scratch bank keep PE busy
            # through the weight-DMA wait so the real burst runs hot
            if n_warm:
                tensor.wait_ge(sOnes, 2)
                for _ in range(n_warm):
                    nc.tensor.matmul(scr[:], ones[:], jnkt[:],
                                     start=True, stop=True)

            sched = [(dmaWa, grp_a), (dmaWb, grp_b), (dmaWc, grp_c)]
            n_total = sum(len(g) for _, g in sched)
            j = 0
            for sem, grp in sched:
                if grp:
                    tensor.wait_ge(sem, 16)
                for (k, ch) in grp:
                    ins = emit(k, ch, j == 0, j == n_total - 1)
                    j += 1
            ins.then_inc(sP, 1)

    nc.compile()
    return nc


def _pack_inputs(x, spline_weight, spline_scaler, bias, grid_points, deg):
    KP = deg
    BQ, OQ = _grid()
    BSH, OSH = B // BQ, O // OQ
    mono = _fit_mono(grid_points.astype(np.float32), deg)        # (deg+1, G)
    A = np.einsum('kg,oig->oik', mono.astype(np.float64),
                  spline_weight.astype(np.float64))              # (O, I, K+1)
    bias_eff = (bias.astype(np.float64) + A[:, :, 0].sum(axis=1))

    WCOLS = KP * 2 * OSH + OSH
    bcol = 3 * 2 * OSH if KP == 5 else (
        4 * 2 * OSH if KP >= 5 else KP * 2 * OSH)
    wrs = []
    for oq in range(OQ):
        osl = slice(oq * OSH, (oq + 1) * OSH)
        wr = np.zeros((128, WCOLS), dtype=np.float32)
        for k in range(1, KP + 1):
            for ch in range(NCH):
                # stationary for (k, ch): [i_in_chunk, o]
                base = ((k - 1) * 2 + ch) * OSH
                if KP == 5:
                    base += OSH if k >= 4 else 0   # bias sits before k4
                else:
                    base += OSH if k >= 5 else 0   # bias sits before k5
                wr[:, base:base + OSH] = A[osl, ch * 128:(ch + 1) * 128, k].T
        be = bias_eff[osl]
        bhi = be.astype(np.float32).astype(np.float16)
        blo = (be - bhi.astype(np.float64)).astype(np.float32)
        wr[0, bcol:bcol + OSH] = bhi.astype(np.float32)
        wr[1, bcol:bcol + OSH] = blo
        wrs.append(wr.astype(np.float16))

    s_row = spline_scaler[0].astype(np.float32)                  # (I,)
    xs_all = (x.astype(np.float32) / s_row[None, :])             # host divide
    in_maps = []
    for c in range(N_CORES):
        bq, oq = divmod(c, OQ)
        xd = xs_all[bq * BSH:(bq + 1) * BSH]                     # (BSH, I)
        xt = xd.T.reshape(NCH, 128, BSH).transpose(1, 0, 2)      # (128,NCH,BSH)
        in_maps.append({"x": np.ascontiguousarray(
                            xt.reshape(128, NCH * BSH)).astype(np.float16),
                        "wr": wrs[oq]})
    return in_maps


LAST_RESULTS = None


def kernel(x, spline_weight, spline_scaler, bias, grid_points):
    global LAST_RESULTS
    x = np.asarray(x, dtype=np.float32)
    spline_weight = np.asarray(spline_weight, dtype=np.float32)
    spline_scaler = np.asarray(spline_scaler, dtype=np.float32)
    bias = np.asarray(bias, dtype=np.float32)
    grid_points = np.asarray(grid_points, dtype=np.float32)

    if (x.shape != (B, I) or spline_weight.shape != (O, I, G)
            or not np.array_equal(spline_scaler,
                                  np.broadcast_to(spline_scaler[0:1, :],
                                                  spline_scaler.shape))):
        return _reference_numpy(x, spline_weight, spline_scaler, bias,
                                grid_points)

    from concourse.bass_utils import run_bass_kernel_spmd

    deg = int(os.environ.get("NKERN_DEG", "5"))
    waitout = bool(int(os.environ.get("NKERN_WAITOUT", "0")))
    key = (deg, waitout, os.environ.get("NKERN_WARM", "7"),
           os.environ.get("NKERN_GRID", "4x2"),
           os.environ.get("NKERN_OUTQ", "sync"),
           os.environ.get("NKERN_BLOCK", "0"))
    if key not in _CACHE:
        _CACHE[key] = _build_program(deg, waitout)
    nc = _CACHE[key]
    in_maps = _pack_inputs(x, spline_weight, spline_scaler, bias,
                           grid_points, deg)

    trace = bool(int(os.environ.get("NKERN_TRACE", "0")))
    if trace:
        _ensure_axon_ntff_hook()
    res = run_bass_kernel_spmd(nc, in_maps, list(range(N_CORES)), trace=trace)
    LAST_RESULTS = res
    BQ, OQ = _grid()
    BSH, OSH = B // BQ, O // OQ
    out = np.empty((B, O), dtype=np.float32)
    for c in range(N_CORES):
        bq, oq = divmod(c, OQ)
        out[bq * BSH:(bq + 1) * BSH, oq * OSH:(oq + 1) * OSH] = \
            res.results[c]["out"]
    return out


# revision 26
# speedup vs baseline: 1.0270x; 1.0270x over previous
"""Trainium2 Bass kernel for EnhancedKANLayer (spline-order-3 KAN layer).

Reference computation (fp32):
    x_norm = tanh(x[:, None, :] / scaler[None, :, :])          # (B, O, I)
    d      = |x_norm[..., None] - grid|                        # (B, O, I, G)
    b      = exp(-d**3);  bhat = b / (sum_g b + 1e-8)
    out    = einsum('boig,oig->bo', bhat, W) + bias

With scaler uniform across O (as produced by setup_inputs), x_norm is
O-independent.  The G=8 normalized basis functions bhat_g(t) are fixed
smooth scalar functions of t = tanh(x) on (-1, 1), so we replace them by
a degree-D polynomial fit (Chebyshev fit, converted to monomial basis;
coefficients are small so monomials are bf16-safe):

    bhat_g(t) ~= sum_k c[k,g] t^k
    out[b,o]  = sum_{i,k} t_{bi}^k A[o,i,k] + bias_eff[o]
    A[o,i,k]  = sum_g c[k,g] W[o,i,g],  bias_eff = bias + sum_i A[:,i,0]

This kills the whole elementwise basis pipeline (sub/abs/square/mult/
exp/reduce/recip/normalize over B*I*G elements) and leaves: one tanh,
a handful of bf16 power products over B*I elements, and K'*2 small
accumulating matmuls.  Fit error at deg 6 gives end-to-end rel ~3.6e-3
(measured against the jax reference; bf16 matmul floor is ~2.4e-3).

Sharding: data-parallel over batch across 8 NeuronCores (64 rows/core,
A replicated).  Per core, raw-bacc program (manual semaphores):
  SYNC:   x DMA (two column-halves = the two i-chunks), out DMA
  SCALAR: weight half-a DMA (k=1,2), ACT table prefetch dummy, tanh
  DVE:    ones memset, x2/x3/x4/P6 bf16 products, psum->sbuf copy
  GPSIMD: weight half-b DMA (k>=3 + bias rows), P5 products
  PE:     13 accumulating bf16 matmuls (K'=6 powers x 2 i-chunks +
          one 2-row Kahan-split bias matmul vs a ones vector)
Bias is applied exactly via the two-row bf16 Kahan split (hi+lo).
Falls back to a pure-numpy reference path if scaler is not uniform
across O or shapes differ (never hit by the real input distribution).
"""

import os
import sys
import types

import numpy as np

N_CORES = 8
B, I, O, G = 512, 256, 128, 8
NCH = I // 128             # i-chunks of 128 partitions (2)
EPS = 1e-8


def _grid():
    """Shard grid: BQ batch-shards x OQ out-shards (BQ*OQ = 8)."""
    g = os.environ.get("NKERN_GRID", "4x2")
    bq, oq = (int(v) for v in g.split("x"))
    assert bq * oq == N_CORES
    return bq, oq

_CACHE = {}
_FIT_CACHE = {}


def _ensure_axon_ntff_hook():
    """Register the NTFF profiling hook (missing antenv.axon_hooks shim).
    Only needed for traced runs; harmless otherwise."""
    try:
        import antenv
        if 'antenv.axon_hooks' not in sys.modules:
            mod = types.ModuleType('antenv.axon_hooks')
            holder = [None]
            mod.set_axon_ntff_profile_hook = lambda h: holder.__setitem__(0, h)
            mod.get_axon_ntff_profile_hook = lambda: holder[0]
            sys.modules['antenv.axon_hooks'] = mod
            antenv.axon_hooks = mod
        mod = sys.modules['antenv.axon_hooks']
        if mod.get_axon_ntff_profile_hook() is None:
            from trn_agent_boot.trn_boot import _ntff_profile_via_ctypes
            so = '/opt/axon/libaxon_pjrt.so'
            if os.path.exists(so):
                mod.set_axon_ntff_profile_hook(_ntff_profile_via_ctypes(so))
    except Exception:
        pass


def _reference_numpy(x, spline_weight, spline_scaler, bias, grid_points):
    """General fallback, mirrors the jax reference in numpy (fp32)."""
    x = x.astype(np.float32)
    xn = np.tanh(x[:, None, :] / spline_scaler[None, :, :])          # (B,O,I)
    d = np.abs(xn[..., None] - grid_points)                           # (B,O,I,G)
    b = np.exp(-(d ** 3))
    bhat = b / (b.sum(axis=-1, keepdims=True) + EPS)
    out = np.einsum('boig,oig->bo', bhat, spline_weight, optimize=True)
    return (out + bias[None, :]).astype(np.float32)


def _fit_mono(grid_points, deg):
    """Chebyshev-fit the G normalized basis functions on t in [-1,1],
    return monomial coefficients mono[k, g] (k = 0..deg)."""
    key = (grid_points.tobytes(), deg)
    if key in _FIT_CACHE:
        return _FIT_CACHE[key]
    import numpy.polynomial.chebyshev as C
    g = grid_points.astype(np.float64)
    ts = np.cos(np.pi * (np.arange(4000) + 0.5) / 4000)
    d = np.abs(ts[:, None] - g[None, :])
    b = np.exp(-(d ** 3))
    bh = b / (b.sum(-1, keepdims=True) + EPS)
    mono = np.stack(
        [C.cheb2poly(C.chebfit(ts, bh[:, j], deg)) for j in range(len(g))],
        axis=1)                                                  # (deg+1, G)
    _FIT_CACHE[key] = mono
    return mono


def _build_program(deg, waitout):
    """Raw bacc program for the polynomial-KAN kernel; deg+1 = K powers.

    Power products (all bf16, halves h = i-chunk):
      T = tanh(x)          [ACT]
      x2 = T*T, x3 = x2*T, x4 = x2*x2, P6 = x3*x3   [DVE]
      P5 = x2*x3           [GPSIMD]
      (deg 7 adds P7 = x3*x4 on GPSIMD; deg 5 drops P6)
    """
    from contextlib import ExitStack

    from concourse import bacc, mybir

    f32 = mybir.dt.float32
    f16 = mybir.dt.float16
    AF = mybir.ActivationFunctionType
    ALU = mybir.AluOpType

    KP = deg                     # number of non-constant powers (k = 1..KP)
    assert 4 <= KP <= 7
    BQ, OQ = _grid()
    BSH, OSH = B // BQ, O // OQ  # per-core batch rows / out cols
    XC = NCH * BSH               # x-tile cols (ch, b)
    WA = 2 * 2 * OSH             # k=1,2 cols (both chunks) -> Sync queue
    if KP == 5:
        # k3 + bias -> Scalar queue; k4,k5 -> GpSimd queue
        WB = WA + 2 * OSH + OSH
    else:
        # k3,k4 + bias -> Scalar queue; k>=5 -> GpSimd queue
        WB = WA + (2 if KP >= 4 else 1) * 2 * OSH + OSH
    WCOLS = KP * 2 * OSH + OSH   # + bias block (2 Kahan rows x OSH o)

    nc = bacc.Bacc("TRN2", target_bir_lowering=False, debug=False,
                   num_devices=N_CORES)

    x_d = nc.dram_tensor("x", [128, XC], f16, kind="ExternalInput")
    wr_d = nc.dram_tensor("wr", [128, WCOLS], f16, kind="ExternalInput")
    out_d = nc.dram_tensor("out", [BSH, OSH], f32, kind="ExternalOutput")

    def wcol(k, ch):
        base = ((k - 1) * 2 + ch) * OSH
        if KP == 5:
            return base + (OSH if k >= 4 else 0)   # bias sits before k4
        return base + (OSH if k >= 5 else 0)       # bias sits before k5

    bias_col = 3 * 2 * OSH if KP == 5 else (
        4 * 2 * OSH if KP >= 5 else KP * 2 * OSH)

    with ExitStack() as ctx:
        e = ctx.enter_context
        xs = e(nc.sbuf_tensor([128, XC], f16))
        T = e(nc.sbuf_tensor([128, XC], f16))
        x2 = e(nc.sbuf_tensor([128, XC], f16))
        x3 = e(nc.sbuf_tensor([128, XC], f16))
        x4 = e(nc.sbuf_tensor([128, XC], f16))
        P5 = e(nc.sbuf_tensor([128, XC], f16))
        P6 = e(nc.sbuf_tensor([128, XC], f16))
        P7 = e(nc.sbuf_tensor([128, XC], f16))
        wr = e(nc.sbuf_tensor([128, WCOLS], f16))
        ones = e(nc.sbuf_tensor([2, BSH], f16))
        jnkt = e(nc.sbuf_tensor([2, 256], f16))
        scr = e(nc.psum_tensor([BSH, 256], f32))
        outsb = e(nc.sbuf_tensor([BSH, OSH], f32))
        psum = e(nc.psum_tensor([BSH, OSH], f32))

        dmaX0 = e(nc.semaphore("dmaX0"))
        dmaX1 = e(nc.semaphore("dmaX1"))
        dmaWa = e(nc.semaphore("dmaWa"))
        dmaWb = e(nc.semaphore("dmaWb"))
        dmaWc = e(nc.semaphore("dmaWc"))
        dmaO = e(nc.semaphore("dmaO"))
        sOnes = e(nc.semaphore("sOnes"))
        sT = e(nc.semaphore("sT"))
        s2 = e(nc.semaphore("s2"))
        s3 = e(nc.semaphore("s3"))
        s4 = e(nc.semaphore("s4"))
        s5 = e(nc.semaphore("s5"))
        s6 = e(nc.semaphore("s6"))
        s7 = e(nc.semaphore("s7"))
        sP = e(nc.semaphore("sP"))
        sC = e(nc.semaphore("sC"))

        def hs(h):
            return slice(h * BSH, (h + 1) * BSH)

        dmaX = (dmaX0, dmaX1)

        n_warm = int(os.environ.get("NKERN_WARM", "7"))
        outq = os.environ.get("NKERN_OUTQ", "sync")
        use_block = bool(int(os.environ.get("NKERN_BLOCK", "0")))

        # All instructions are emitted flat into the main bb (no Block):
        # no per-engine entry branches, no exit drains, no exit barrier —
        # the NRT epilogue synchronizes and drains anyway.
        nc.sync.dma_start(xs[:, hs(0)], x_d.ap()[:, hs(0)]).then_inc(dmaX0, 16)
        nc.scalar.dma_start(xs[:, hs(1)],
                            x_d.ap()[:, hs(1)]).then_inc(dmaX1, 16)
        nc.sync.dma_start(wr[:, 0:WA], wr_d.ap()[:, 0:WA]).then_inc(dmaWa, 16)
        nc.scalar.dma_start(wr[:, WA:WB],
                            wr_d.ap()[:, WA:WB]).then_inc(dmaWb, 16)
        nc.gpsimd.dma_start(wr[:, WB:WCOLS],
                            wr_d.ap()[:, WB:WCOLS]).then_inc(dmaWc, 16)

        if use_block:
            block = e(nc.Block(no_gpsimd_drain=True))
            sync_sec = block.sync
            scalar_sec = block.scalar
            vector_sec = block.vector
            gpsimd_sec = block.gpsimd
            tensor_sec = block.tensor
        else:
            def _flat(f):
                return lambda g: g(f)
            sync_sec = lambda f: f(nc.sync)
            scalar_sec = lambda f: f(nc.scalar)
            vector_sec = lambda f: f(nc.vector)
            gpsimd_sec = lambda f: f(nc.gpsimd)
            tensor_sec = lambda f: f(nc.tensor)

        @sync_sec
        def _(sync):
            if outq == "sync":
                sync.wait_ge(sC, 1)
                sync.dma_start(out_d.ap(), outsb[:]).then_inc(dmaO, 16)
                if waitout:
                    sync.wait_ge(dmaO, 16)

        @scalar_sec
        def _(scalar):
            # (no dummy ACT needed: the ACT_TABLE_LOAD pseudo-inst hoists to
            # the scalar stream start, well before x lands)
            for h in range(2):
                scalar.wait_ge(dmaX[h], 16)
                nc.scalar.activation(T[:, hs(h)], xs[:, hs(h)],
                                     AF.Tanh).then_inc(sT, 1)
            if KP == 5:
                scalar.wait_ge(s2, 1)
                nc.scalar.activation(x4[:, hs(0)], x2[:, hs(0)],
                                     AF.Square).then_inc(s4, 1)

        @vector_sec
        def _(vector):
            nc.vector.memset(ones[:], 1.0).then_inc(sOnes, 1)
            if n_warm:
                nc.vector.memset(jnkt[:], 0.5).then_inc(sOnes, 1)
            if KP == 5:
                for h in range(2):
                    vector.wait_ge(sT, h + 1)
                    nc.vector.tensor_tensor(x2[:, hs(h)], T[:, hs(h)],
                                            T[:, hs(h)],
                                            op=ALU.mult).then_inc(s2, 1)
                    nc.vector.tensor_tensor(x3[:, hs(h)], x2[:, hs(h)],
                                            T[:, hs(h)],
                                            op=ALU.mult).then_inc(s3, 1)
                for h in range(2):
                    nc.vector.tensor_tensor(P5[:, hs(h)], x2[:, hs(h)],
                                            x3[:, hs(h)],
                                            op=ALU.mult).then_inc(s5, 1)
            else:
                for h in range(2):
                    vector.wait_ge(sT, h + 1)
                    nc.vector.tensor_tensor(x2[:, hs(h)], T[:, hs(h)],
                                            T[:, hs(h)],
                                            op=ALU.mult).then_inc(s2, 1)
                    nc.vector.tensor_tensor(x3[:, hs(h)], x2[:, hs(h)],
                                            T[:, hs(h)],
                                            op=ALU.mult).then_inc(s3, 1)
                    if KP >= 4:
                        nc.vector.tensor_tensor(x4[:, hs(h)], x2[:, hs(h)],
                                                x2[:, hs(h)],
                                                op=ALU.mult).then_inc(s4, 1)
                    if KP >= 6:
                        nc.vector.tensor_tensor(P6[:, hs(h)], x3[:, hs(h)],
                                                x3[:, hs(h)],
                                                op=ALU.mult).then_inc(s6, 1)
            vector.wait_ge(sP, 1)
            nc.vector.tensor_scalar(outsb[:], psum[:], 1.0, None,
                                    op0=ALU.mult).then_inc(sC, 1)

        @gpsimd_sec
        def _(gpsimd):
            if KP == 5:
                gpsimd.wait_ge(s2, 2)
                nc.gpsimd.tensor_tensor(x4[:, hs(1)], x2[:, hs(1)],
                                        x2[:, hs(1)],
                                        op=ALU.mult).then_inc(s7, 1)
            else:
                for h in range(2):
                    if KP >= 5:
                        gpsimd.wait_ge(s3, h + 1)
                        nc.gpsimd.tensor_tensor(P5[:, hs(h)], x2[:, hs(h)],
                                                x3[:, hs(h)],
                                                op=ALU.mult).then_inc(s5, 1)
                    if KP >= 7:
                        gpsimd.wait_ge(s4, h + 1)
                        nc.gpsimd.tensor_tensor(P7[:, hs(h)], x3[:, hs(h)],
                                                x4[:, hs(h)],
                                                op=ALU.mult).then_inc(s7, 1)

        pw = {1: T, 2: x2, 3: x3, 4: x4, 5: P5, 6: P6, 7: P7}
        if KP == 5:
            # x4_0 from ACT (s4), x4_1 from GPSIMD (s7): separate sems
            psem = {(1, 0): (sT, 1), (1, 1): (sT, 2),
                    (2, 0): (s2, 1), (2, 1): (s2, 2),
                    (3, 0): (s3, 1), (3, 1): (s3, 2),
                    (4, 0): (s4, 1), (4, 1): (s7, 1),
                    (5, 0): (s5, 1), (5, 1): (s5, 2)}
        else:
            psem = {(k, ch): (s, ch + 1) for k, s in
                    {1: sT, 2: s2, 3: s3, 4: s4, 5: s5, 6: s6, 7: s7}.items()
                    for ch in range(2)}

        @tensor_sec
        def _(tensor):
            # three weight groups, each consumed in chunk-then-k order as
            # its DMA lands; k=0 denotes the 2-row Kahan bias matmul
            grp_a = [(1, 0), (1, 1), (2, 0), (2, 1)]
            if KP == 5:
                # k4 last: its second half comes from the slow GPSIMD mult
                grp_b = [(3, 0), (3, 1), (0, 0)]
                grp_c = [(5, 0), (5, 1), (4, 0), (4, 1)]
            else:
                grp_b = ([(k, ch) for k in range(3, min(KP, 4) + 1)
                          for ch in range(2)] + [(0, 0)])
                grp_c = [(k, ch) for k in range(5, KP + 1)
                         for ch in range(2)]
            waited = {}

            def emit(k, ch, start, stop):
                if k == 0:
                    tensor.wait_ge(sOnes, 1)
                    return nc.tensor.matmul(
                        psum[:], ones[:], wr[0:2, bias_col:bias_col + OSH],
                        start=start, stop=stop)
                sem, need = psem[(k, ch)]
                if waited.get(id(sem), 0) < need:
                    tensor.wait_ge(sem, need)
                    waited[id(sem)] = need
                return nc.tensor.matmul(
                    psum[:], pw[k][:, hs(ch)],
                    wr[:, wcol(k, ch):wcol(k, ch) + OSH],
                    start=start, stop=stop)

            # p-state warmup: junk matmuls on a scratch bank keep PE busy
            # through the weight-DMA wait so the real burst runs hot
            if n_warm:
                tensor.wait_ge(sOnes, 2)
                for _ in range(n_warm):
                    nc.tensor.matmul(scr[:], ones[:], jnkt[:],
                                     start=True, stop=True)

            sched = [(dmaWa, grp_a), (dmaWb, grp_b), (dmaWc, grp_c)]
            n_total = sum(len(g) for _, g in sched)
            j = 0
            for sem, grp in sched:
                if grp:
                    tensor.wait_ge(sem, 16)
                for (k, ch) in grp:
                    ins = emit(k, ch, j == 0, j == n_total - 1)
                    j += 1
            ins.then_inc(sP, 1)

    nc.compile()
    return nc


def _pack_inputs(x, spline_weight, spline_scaler, bias, grid_points, deg):
    KP = deg
    BQ, OQ = _grid()
    BSH, OSH = B // BQ, O // OQ
    mono = _fit_mono(grid_points.astype(np.float32), deg)        # (deg+1, G)
    A = np.einsum('kg,oig->oik', mono.astype(np.float64),
                  spline_weight.astype(np.float64))              # (O, I, K+1)
    bias_eff = (bias.astype(np.float64) + A[:, :, 0].sum(axis=1))

    WCOLS = KP * 2 * OSH + OSH
    bcol = 3 * 2 * OSH if KP == 5 else (
        4 * 2 * OSH if KP >= 5 else KP * 2 * OSH)
    wrs = []
    for oq in range(OQ):
        osl = slice(oq * OSH, (oq + 1) * OSH)
        wr = np.zeros((128, WCOLS), dtype=np.float32)
        for k in range(1, KP + 1):
            for ch in range(NCH):
                # stationary for (k, ch): [i_in_chunk, o]
                base = ((k - 1) * 2 + ch) * OSH
                if KP == 5:
                    base += OSH if k >= 4 else 0   # bias sits before k4
                else:
                    base += OSH if k >= 5 else 0   # bias sits before k5
                wr[:, base:base + OSH] = A[osl, ch * 128:(ch + 1) * 128, k].T
        be = bias_eff[osl]
        bhi = be.astype(np.float32).astype(np.float16)
        blo = (be - bhi.astype(np.float64)).astype(np.float32)
        wr[0, bcol:bcol + OSH] = bhi.astype(np.float32)
        wr[1, bcol:bcol + OSH] = blo
        wrs.append(wr.astype(np.float16))

    s_row = spline_scaler[0].astype(np.float32)                  # (I,)
    xs_all = (x.astype(np.float32) / s_row[None, :])             # host divide
    in_maps = []
    for c in range(N_CORES):
        bq, oq = divmod(c, OQ)
        xd = xs_all[bq * BSH:(bq + 1) * BSH]                     # (BSH, I)
        xt = xd.T.reshape(NCH, 128, BSH).transpose(1, 0, 2)      # (128,NCH,BSH)
        in_maps.append({"x": np.ascontiguousarray(
                            xt.reshape(128, NCH * BSH)).astype(np.float16),
                        "wr": wrs[oq]})
    return in_maps


LAST_RESULTS = None


def kernel(x, spline_weight, spline_scaler, bias, grid_points):
    global LAST_RESULTS
    x = np.asarray(x, dtype=np.float32)
    spline_weight = np.asarray(spline_weight, dtype=np.float32)
    spline_scaler = np.asarray(spline_scaler, dtype=np.float32)
    bias = np.asarray(bias, dtype=np.float32)
    grid_points = np.asarray(grid_points, dtype=np.float32)

    if (x.shape != (B, I) or spline_weight.shape != (O, I, G)
            or not np.array_equal(spline_scaler,
                                  np.broadcast_to(spline_scaler[0:1, :],
                                                  spline_scaler.shape))):
        return _reference_numpy(x, spline_weight, spline_scaler, bias,
                                grid_points)

    from concourse.bass_utils import run_bass_kernel_spmd

    deg = int(os.environ.get("NKERN_DEG", "5"))
    waitout = bool(int(os.environ.get("NKERN_WAITOUT", "0")))
    key = (deg, waitout, os.environ.get("NKERN_WARM", "7"),
           os.environ.get("NKERN_GRID", "4x2"),
           os.environ.get("NKERN_OUTQ", "sync"),
           os.environ.get("NKERN_BLOCK", "0"))
    if key not in _CACHE:
        _CACHE[key] = _build_program(deg, waitout)
    nc = _CACHE[key]
    in_maps = _pack_inputs(x, spline_weight, spline_scaler, bias,
                           grid_points, deg)

    trace = bool(int(os.environ.get("NKERN_TRACE", "0")))
    if trace:
        _ensure_axon_ntff_hook()
    res = run_bass_kernel_spmd(nc, in_maps, list(range(N_CORES)), trace=trace)
    LAST_RESULTS = res
    BQ, OQ = _grid()
    BSH, OSH = B // BQ, O // OQ
    out = np.empty((B, O), dtype=np.float32)
    for c in range(N_CORES):
        bq, oq = divmod(c, OQ)
        out[bq * BSH:(bq + 1) * BSH, oq * OSH:(oq + 1) * OSH] = \
            res.results[c]["out"]
    return out
